# revision 8
# baseline (speedup 1.0000x reference)
"""Trainium2 Bass kernel: per-batch segment-mean pooling + 3-layer MLP.

Reference computation (B=64, T=512, H=768, S=128):
  pooled[b,s,:] = mean over t of hidden[b,t,:] where statements_ids[b,t]==s
  x = gelu(pooled @ w1 + b1); x = gelu(x @ w2 + b2)
  out[b,s] = sigmoid(x @ w3 + b3)

Distribution: data-parallel over batch across 8 NeuronCores (8 batches per
core); MLP weights replicated.

Per-core algorithm (all matmuls on PE at fp32r 1 cycle/row):
  - Build the one-hot matrix MT[t,s] = (sid[t]==s) on DVE via
    tensor_scalar(is_equal) against an iota constant.
  - counts = MT.T @ ones            (PE)        -> inv = 1/max(counts,1) (DVE)
  - pooled_sums = MT.T @ hidden[b]  (PE, [S,H]) -> pooled = sums*inv     (DVE)
  - X^T tiles via PE transpose (pooled is [S,H] but the MLP wants [H, rows])
  - MLP batched over all 8 local batches: rows = 8*128 = 1024 moving dim,
    weights stationary; gelu/sigmoid + bias fused on ACT.
"""

import os
import sys

sys.path.insert(0, "/opt/trn_rl_repo")

import numpy as np

import concourse.bass as bass
import concourse.mybir as mybir
import concourse.tile as tile
from concourse import bacc, bass_utils

B, T, H, S = 64, 512, 768, 128
N_CORES = 8
BL = B // N_CORES  # local batches per core
P = 128
KT = T // P        # t-tiles per batch
KH = H // P        # h-tiles
R = BL * S         # MLP rows per core
RC = 2 * S         # moving-dim chunk (2 batches) -- >=256 keeps fp32r at 1 cyc/row
NRC = R // RC
HC = H // 2        # pooled psum chunk (384 <= 512 fits one PSUM bank)

_CACHE: dict = {}


def _build_program(act_func=None):
    f32, f32r, i32 = mybir.dt.float32, mybir.dt.float32r, mybir.dt.int32
    FT = mybir.ActivationFunctionType
    OP = mybir.AluOpType

    nc = bacc.Bacc("TRN2", target_bir_lowering=False, debug=False)
    hid = nc.dram_tensor("hidden", [BL, T, H], f32r, kind="ExternalInput").ap()
    sid = nc.dram_tensor("sid", [BL, T], i32, kind="ExternalInput").ap()
    w1 = nc.dram_tensor("w1", [H, H], f32r, kind="ExternalInput").ap()
    w2 = nc.dram_tensor("w2", [H, H], f32r, kind="ExternalInput").ap()
    w3 = nc.dram_tensor("w3", [H, 1], f32r, kind="ExternalInput").ap()
    b1 = nc.dram_tensor("b1", [H], f32, kind="ExternalInput").ap()
    b2 = nc.dram_tensor("b2", [H], f32, kind="ExternalInput").ap()
    b3 = nc.dram_tensor("b3", [1], f32, kind="ExternalInput").ap()
    iota = nc.dram_tensor("iota", [P, P], f32, kind="ExternalInput").ap()
    ident = nc.dram_tensor("ident", [P, P], f32, kind="ExternalInput").ap()
    ones = nc.dram_tensor("ones", [P, 1], f32, kind="ExternalInput").ap()
    out = nc.dram_tensor("out", [BL, S], f32, kind="ExternalOutput").ap()

    with tile.TileContext(nc) as tc:
        with (
            tc.tile_pool(name="consts", bufs=1) as consts,
            tc.tile_pool(name="wpool", bufs=1) as wpool,
            tc.tile_pool(name="hpool", bufs=3) as hpool,
            tc.tile_pool(name="mtpool", bufs=8) as mtpool,
            tc.tile_pool(name="small", bufs=3) as small,
            tc.tile_pool(name="xtpool", bufs=1) as xtpool,
            tc.tile_pool(name="ypool", bufs=1) as ypool,
            tc.tile_pool(name="ps", bufs=8, space="PSUM") as ps,
        ):
            iota_sb = consts.tile([P, P], f32)
            nc.sync.dma_start(iota_sb, iota)
            ident_sb = consts.tile([P, P], f32)
            nc.sync.dma_start(ident_sb, ident)
            ones_sb = consts.tile([P, 1], f32)
            nc.sync.dma_start(ones_sb, ones)
            sid_sb = consts.tile([P, BL, KT], i32)
            nc.sync.dma_start(sid_sb, sid.rearrange("b (k p) -> p b k", p=P))
            b1_sb = consts.tile([P, KH], f32)
            nc.sync.dma_start(b1_sb, b1.rearrange("(m p) -> p m", p=P))
            b2_sb = consts.tile([P, KH], f32)
            nc.sync.dma_start(b2_sb, b2.rearrange("(m p) -> p m", p=P))
            b3_sb = consts.tile([1, 1], f32)
            nc.sync.dma_start(b3_sb, b3.rearrange("(a o) -> a o", a=1))
            w3_sb = consts.tile([P, KH], f32r)
            nc.sync.dma_start(w3_sb, w3.rearrange("(k p) o -> p (k o)", p=P))

            w1_sb = wpool.tile([P, KH, H], f32r, tag="w1")
            nc.sync.dma_start(w1_sb, w1.rearrange("(k p) j -> p k j", p=P))
            w2_sb = wpool.tile([P, KH, H], f32r, tag="w2")
            nc.sync.dma_start(w2_sb, w2.rearrange("(k p) j -> p k j", p=P))

            xts = [xtpool.tile([P, R], f32r, tag=f"xt{k}", name=f"xt{k}") for k in range(KH)]
            y1s = [ypool.tile([P, R], f32r, tag=f"y1_{m}", name=f"y1_{m}") for m in range(KH)]
            y2s = [ypool.tile([P, R], f32r, tag=f"y2_{m}", name=f"y2_{m}") for m in range(KH)]
            pred = ypool.tile([1, R], f32, tag="pred")

            def pool_batch(b):
                hb = hpool.tile([P, KT, H], f32r, tag="hb")
                nc.sync.dma_start(hb, hid[b].rearrange("(k p) h -> p k h", p=P))
                sidf = small.tile([P, KT], f32, tag="sidf")
                nc.vector.tensor_copy(sidf, sid_sb[:, b, :])
                cnt = ps.tile([P, 1], f32, tag="ps")
                mts = []
                for k in range(KT):
                    mt = mtpool.tile([P, P], f32r, tag="mt")
                    nc.vector.tensor_tensor(
                        mt,
                        iota_sb,
                        sidf[:, k : k + 1].to_broadcast((P, P)),
                        OP.is_equal,
                    )
                    nc.tensor.matmul(
                        cnt,
                        lhsT=mt.bitcast(f32),
                        rhs=ones_sb,
                        start=(k == 0),
                        stop=(k == KT - 1),
                    )
                    mts.append(mt)
                inv = small.tile([P, 1], f32, tag="inv")
                nc.vector.tensor_scalar(inv, cnt, 1.0, None, OP.max)
                nc.vector.reciprocal(inv, inv)
                pooled = small.tile([P, H], f32, tag="pooled")
                for nch in range(2):
                    pp = ps.tile([P, HC], f32, tag="ps")
                    for k in range(KT):
                        nc.tensor.matmul(
                            pp,
                            lhsT=mts[k],
                            rhs=hb[:, k, nch * HC : (nch + 1) * HC],
                            start=(k == 0),
                            stop=(k == KT - 1),
                        )
                    nc.vector.tensor_tensor(
                        pooled[:, nch * HC : (nch + 1) * HC],
                        pp,
                        inv[:, 0:1].to_broadcast((P, HC)),
                        OP.mult,
                    )
                for m in range(KH):
                    trp = ps.tile([P, P], f32, tag="ps")
                    nc.tensor.transpose(trp, pooled[:, m * P : (m + 1) * P], ident_sb)
                    nc.vector.tensor_copy(xts[m][:, b * S : (b + 1) * S], trp)

            def fc(w_sb, b_sb, xs, outs, rc, func):
                for m in range(KH):
                    pt = ps.tile([P, RC], f32, tag="ps")
                    for k in range(KH):
                        nc.tensor.matmul(
                            pt,
                            lhsT=w_sb[:, k, m * P : (m + 1) * P],
                            rhs=xs[k][:, rc * RC : (rc + 1) * RC],
                            start=(k == 0),
                            stop=(k == KH - 1),
                        )
                    nc.scalar.activation(
                        outs[m][:, rc * RC : (rc + 1) * RC],
                        pt,
                        func,
                        bias=b_sb[:, m : m + 1],
                    )

            def fc3(rc):
                pt = ps.tile([1, RC], f32, tag="ps")
                for k in range(KH):
                    nc.tensor.matmul(
                        pt,
                        lhsT=w3_sb[:, k : k + 1],
                        rhs=y2s[k][:, rc * RC : (rc + 1) * RC],
                        start=(k == 0),
                        stop=(k == KH - 1),
                    )
                nc.scalar.activation(
                    pred[:, rc * RC : (rc + 1) * RC],
                    pt,
                    mybir.ActivationFunctionType.Sigmoid,
                    bias=b3_sb,
                )

            FT = mybir.ActivationFunctionType
            gelu = FT.Gelu if act_func is None else act_func
            for i in range(NRC):
                if i >= 1:
                    fc(w2_sb, b2_sb, y1s, y2s, i - 1, gelu)
                pool_batch(2 * i)
                pool_batch(2 * i + 1)
                fc(w1_sb, b1_sb, xts, y1s, i, gelu)
            fc(w2_sb, b2_sb, y1s, y2s, NRC - 1, gelu)
            for i in range(NRC):
                fc3(i)
            nc.sync.dma_start(out.rearrange("b s -> (b s)"), pred)

    nc.compile()
    return nc


def _get_program():
    if "nc" not in _CACHE:
        _CACHE["nc"] = _build_program()
    return _CACHE["nc"]


def _consts():
    iota = np.broadcast_to(np.arange(P, dtype=np.float32), (P, P)).copy()
    ident = np.eye(P, dtype=np.float32)
    ones = np.ones((P, 1), dtype=np.float32)
    return iota, ident, ones


def make_in_maps(hidden, statements_ids, w1, b1, w2, b2, w3, b3):
    iota, ident, ones = _consts()
    hidden = np.ascontiguousarray(np.asarray(hidden, dtype=np.float32))
    sid = np.ascontiguousarray(np.asarray(statements_ids, dtype=np.int32))
    in_maps = []
    for c in range(N_CORES):
        in_maps.append(
            {
                "hidden": hidden[c * BL : (c + 1) * BL],
                "sid": sid[c * BL : (c + 1) * BL],
                "w1": np.asarray(w1, dtype=np.float32),
                "w2": np.asarray(w2, dtype=np.float32),
                "w3": np.asarray(w3, dtype=np.float32),
                "b1": np.asarray(b1, dtype=np.float32),
                "b2": np.asarray(b2, dtype=np.float32),
                "b3": np.asarray(b3, dtype=np.float32),
                "iota": iota,
                "ident": ident,
                "ones": ones,
            }
        )
    return in_maps


def kernel(hidden, statements_ids, w1, b1, w2, b2, w3, b3, **kwargs):
    nc = _get_program()
    in_maps = make_in_maps(hidden, statements_ids, w1, b1, w2, b2, w3, b3)
    trace = bool(int(os.environ.get("KERNEL_TRACE", "0")))
    res = bass_utils.run_bass_kernel_spmd(
        nc, in_maps, core_ids=list(range(N_CORES)), trace=trace
    )
    _CACHE["last_results"] = res
    out = np.concatenate([res.results[c]["out"] for c in range(N_CORES)], axis=0)
    return out.astype(np.float32)
